# revision 2
# baseline (speedup 1.0000x reference)
"""Distributed Trainium2 Bass kernel for GQA attention prefill — v2.

Problem: B=2, S=2048, D=4096, 32 q heads, 8 kv heads, head_dim=128, RoPE,
causal mask, start_pos=0.

Sharding (8 cores): DP2 over batch x TP4 over heads.  Core c = b*4 + g gets
batch b, q-heads 8g..8g+7, kv-heads 2g..2g+1, wo rows for those q-heads.
Each core computes a partial [S, D] output; the host sums the 4 partials
per batch (the row-parallel wo unshard).

v2 changes vs v1:
  - x arrives pre-transposed AND pre-cast to bf16 from the host (xt param),
    eliminating all on-device x casts/bounces/PE-transposes.
  - weights and wo arrive bf16 (host cast), halving weight DMA.
  - cos/sin RoPE tables arrive pre-built in on-chip layout [128, S] bf16.
  - causal mask add narrowed to the 128-wide diagonal block.
  - reciprocal reads the PSUM rowsum directly (no staging copy).
  - wo-projection matmuls are interleaved into the attention instruction
    stream as fillers, hiding the exp (ACT) dependency gaps.
"""

import math
from collections import deque

import numpy as np

import concourse.bass as bass  # noqa: F401  (bass types via bacc)
import concourse.mybir as mybir
from concourse import bacc
from concourse.bass_utils import run_bass_kernel_spmd
from concourse.tile import TileContext

F32 = mybir.dt.float32
BF16 = mybir.dt.bfloat16

B, S, D = 2, 2048, 4096
NH, NKV, HD = 32, 8, 128
NCORES = 8
TPG = 4                  # tensor-parallel groups
NQL = NH // TPG          # 8 local q heads
NKVL = NKV // TPG        # 2 local kv heads
SCW = 512                # s-chunk width
NSC = S // SCW           # 4 s-chunks
NKC = D // 128           # 32 contraction chunks for projections
NTC = S // 128           # 16 T-chunks (key positions)
HW = S // 2              # half width (1024)
NM = NQL + 2 * NKVL      # 12 projection m-chunks
WBLK = NKC * HD          # weight cols per m-chunk
SCALE = 1.0 / math.sqrt(HD)
NEG = -1e9


def _build():
    nc = bacc.Bacc("TRN2", target_bir_lowering=False, debug=False,
                   num_devices=NCORES)
    # x^T pre-transposed+bf16: [128, sc-major(4) x kc-major(32) x 512]
    xt_d = nc.declare_dram_parameter("xt", [128, NSC * NKC * SCW], BF16,
                                     isOutput=False)
    # weights pre-tiled bf16: [128, m-major kc-major cols]
    wq = nc.declare_dram_parameter("wq", [128, NQL * WBLK], BF16, isOutput=False)
    wk = nc.declare_dram_parameter("wk", [128, NKVL * WBLK], BF16, isOutput=False)
    wv = nc.declare_dram_parameter("wv", [128, NKVL * WBLK], BF16, isOutput=False)
    # wo pre-tiled bf16: [128, dc-major(8) x m-major(8) x 512]
    wo = nc.declare_dram_parameter("wo", [128, (D // SCW) * NQL * SCW], BF16,
                                   isOutput=False)
    # RoPE tables pre-built in on-chip layout
    cos2_d = nc.declare_dram_parameter("cos2", [128, S], BF16, isOutput=False)
    sin2n_d = nc.declare_dram_parameter("sin2n", [128, S], BF16, isOutput=False)
    out = nc.declare_dram_parameter("out", [S, D], BF16, isOutput=True)

    with TileContext(nc) as tc:
        with (
            tc.tile_pool(name="const", bufs=1) as const,
            tc.tile_pool(name="big", bufs=1) as big,
            tc.tile_pool(name="sb", bufs=3) as sb,
            tc.tile_pool(name="ps", bufs=1, space="PSUM") as ps,
        ):
            # ---- constants ------------------------------------------------
            ident = const.tile([128, 128], BF16, name="ident")
            nc.gpsimd.memset(ident[:], 0.0)
            nc.gpsimd.affine_select(
                out=ident[:], in_=ident[:],
                compare_op=mybir.AluOpType.not_equal, fill=1.0,
                base=0, pattern=[[-1, 128]], channel_multiplier=1,
            )
            ones = const.tile([128, 128], BF16, name="ones")
            nc.gpsimd.memset(ones[:], 1.0)
            # causal mask for the 128-wide diagonal block: keep where c >= p
            maskdiag = const.tile([128, 128], F32, name="maskdiag")
            nc.gpsimd.memset(maskdiag[:], 0.0)
            nc.gpsimd.affine_select(
                out=maskdiag[:], in_=maskdiag[:],
                compare_op=mybir.AluOpType.is_ge, fill=NEG,
                base=0, pattern=[[1, 128]], channel_multiplier=-1,
            )
            cos2 = const.tile([128, S], BF16, name="cos2")
            sin2n = const.tile([128, S], BF16, name="sin2n")
            nc.scalar.dma_start(out=cos2[:], in_=cos2_d[:, :])
            nc.scalar.dma_start(out=sin2n[:], in_=sin2n_d[:, :])

            ksb = big.tile([128, NKVL * S], BF16, name="ksb")
            vsb = big.tile([128, NTC * NKVL * HD], BF16, name="vsb")

            # per-(hf, head) tiles
            qt = {}
            attnT = {}

            # ---- wo filler machinery -------------------------------------
            pending = deque()

            def wo_gen(hf):
                """Output projection for s rows [hf*1024, (hf+1)*1024)."""
                for dc in range(D // SCW):
                    wot = sb.tile([128, NQL * SCW], BF16,
                                  name=f"wot{hf}_{dc}", tag="wot", bufs=2)
                    nc.scalar.dma_start(
                        out=wot[:],
                        in_=wo[:, dc * NQL * SCW : (dc + 1) * NQL * SCW])
                    for ssub in range(HW // 128):
                        pd = ps.tile([128, SCW], F32, name=f"pd{hf}_{dc}_{ssub}",
                                     tag="pp", bufs=2)
                        for kc8 in range(NQL):
                            nc.tensor.matmul(
                                pd[:],
                                attnT[(hf, kc8)][:, ssub * 128 : (ssub + 1) * 128],
                                wot[:, kc8 * SCW : (kc8 + 1) * SCW],
                                start=(kc8 == 0), stop=(kc8 == NQL - 1))
                        os_ = sb.tile([128, SCW], BF16, name=f"os{hf}_{dc}_{ssub}",
                                      tag="os", bufs=4)
                        nc.scalar.copy(out=os_[:], in_=pd[:])
                        nc.sync.dma_start(
                            out=out[hf * HW + ssub * 128 : hf * HW + (ssub + 1) * 128,
                                    dc * SCW : (dc + 1) * SCW],
                            in_=os_[:])
                        yield

            def draw(n):
                for _ in range(n):
                    while pending:
                        try:
                            next(pending[0])
                            break
                        except StopIteration:
                            pending.popleft()
                    else:
                        return

            def drain_all():
                while pending:
                    for _ in pending.popleft():
                        pass

            # ---- main schedule -------------------------------------------
            def _wsrc(m):
                if m < NQL:
                    return wq[:, m * WBLK : (m + 1) * WBLK]
                if m < NQL + NKVL:
                    return wk[:, (m - NQL) * WBLK : (m - NQL + 1) * WBLK]
                return wv[:, (m - NQL - NKVL) * WBLK
                          : (m - NQL - NKVL + 1) * WBLK]

            for hf in range(2):
                morder = list(range(NQL, NM)) + list(range(NQL))
                # prefetch the first two weight slices BEFORE the bulky xT
                # DMAs so the first matmul isn't head-of-line blocked
                wpre = {}
                for m in morder[:2]:
                    w = sb.tile([128, WBLK], BF16, name=f"w{hf}_{m}",
                                tag="wsl", bufs=2)
                    nc.sync.dma_start(out=w[:], in_=_wsrc(m))
                    wpre[m] = w

                # xT tiles for this half (4x 1MB DMA chunks each)
                xts = {}
                for scq in range(2):
                    sc = hf * 2 + scq
                    t = sb.tile([128, NKC * SCW], BF16, name=f"xt{sc}",
                                tag="xt", bufs=2)
                    for g in range(4):
                        nc.sync.dma_start(
                            out=t[:, g * 8 * SCW : (g + 1) * 8 * SCW],
                            in_=xt_d[:, sc * NKC * SCW + g * 8 * SCW
                                     : sc * NKC * SCW + (g + 1) * 8 * SCW])
                    xts[scq] = t

                # ---- QKV projection (K,V first, then Q) ------------------
                for m in morder:
                    if m in wpre:
                        wsl = wpre[m]
                    else:
                        wsl = sb.tile([128, WBLK], BF16, name=f"w{hf}_{m}",
                                      tag="wsl", bufs=2)
                        nc.sync.dma_start(out=wsl[:], in_=_wsrc(m))
                    if m < NQL and (hf, m) not in qt:
                        qt[(hf, m)] = sb.tile([128, HW], BF16, name=f"q{hf}_{m}",
                                              tag=f"q{m}", bufs=1)
                    for scq in range(2):
                        sc = hf * 2 + scq
                        ssl = slice(sc * SCW, (sc + 1) * SCW)
                        pp = ps.tile([128, SCW], F32, name=f"pp{hf}_{m}_{scq}",
                                     tag="pp", bufs=2)
                        for kc in range(NKC):
                            nc.tensor.matmul(
                                pp[:], wsl[:, kc * 128 : (kc + 1) * 128],
                                xts[scq][:, kc * SCW : (kc + 1) * SCW],
                                start=(kc == 0), stop=(kc == NKC - 1),
                            )
                        if m < NQL + NKVL:
                            # RoPE -> Q tile or K store
                            if m < NQL:
                                dst = qt[(hf, m)][:, scq * SCW : (scq + 1) * SCW]
                            else:
                                kv = m - NQL
                                dst = ksb[:, kv * S + sc * SCW
                                          : kv * S + (sc + 1) * SCW]
                            t1 = sb.tile([128, SCW], BF16, name=f"t1_{hf}_{m}_{scq}",
                                         tag="t1", bufs=2)
                            t2 = sb.tile([128, SCW], BF16, name=f"t2_{hf}_{m}_{scq}",
                                         tag="t2", bufs=2)
                            nc.vector.tensor_tensor(
                                out=t1[0:64, :], in0=pp[64:128, :],
                                in1=sin2n[0:64, ssl], op=mybir.AluOpType.mult)
                            nc.vector.tensor_tensor(
                                out=t1[64:128, :], in0=pp[0:64, :],
                                in1=sin2n[64:128, ssl], op=mybir.AluOpType.mult)
                            nc.vector.tensor_tensor(
                                out=t2[:], in0=pp[:], in1=cos2[:, ssl],
                                op=mybir.AluOpType.mult)
                            nc.vector.tensor_tensor(
                                out=dst, in0=t1[:], in1=t2[:],
                                op=mybir.AluOpType.add)
                        else:
                            # V: copy + PE-transpose into vsb slots
                            kv = m - NQL - NKVL
                            vts = sb.tile([128, SCW], BF16, name=f"vts{hf}_{kv}_{scq}",
                                          tag="vts", bufs=2)
                            nc.vector.tensor_copy(out=vts[:], in_=pp[:])
                            for j in range(SCW // 128):
                                pv = ps.tile([128, 128], BF16,
                                             name=f"pv{hf}_{kv}_{scq}_{j}",
                                             tag="pp", bufs=2)
                                nc.tensor.transpose(
                                    pv[:], vts[:, j * 128 : (j + 1) * 128], ident[:])
                                slot = (sc * 4 + j) * NKVL + kv
                                nc.vector.tensor_copy(
                                    out=vsb[:, slot * HD : (slot + 1) * HD],
                                    in_=pv[:])

                # ---- attention for both s-chunks of this half ------------
                for scq in range(2):
                    sc = hf * 2 + scq
                    ntc = 4 * sc + 4
                    for h in range(NQL):
                        if (hf, h) not in attnT:
                            attnT[(hf, h)] = sb.tile(
                                [128, HW], BF16, name=f"at{hf}_{h}",
                                tag=f"at{h}", bufs=2)
                        kv = h // (NQL // NKVL)
                        po = ps.tile([128, SCW], F32, name=f"po{sc}_{h}",
                                     tag="po", bufs=2)
                        pr = ps.tile([128, SCW], F32, name=f"pr{sc}_{h}",
                                     tag="pr", bufs=1)
                        for tcx in range(ntc):
                            j = tcx - 4 * sc
                            off = j * 128 if j > 0 else 0
                            w = SCW - off
                            qs0 = scq * SCW + off
                            pss = ps.tile([128, SCW], F32,
                                          name=f"ps{sc}_{h}_{tcx}", tag="sc", bufs=3)
                            nc.tensor.matmul(
                                pss[:, :w],
                                ksb[:, kv * S + tcx * 128 : kv * S + (tcx + 1) * 128],
                                qt[(hf, h)][:, qs0 : qs0 + w],
                                start=True, stop=True,
                            )
                            if j >= 0:
                                nc.vector.tensor_tensor(
                                    out=pss[:, 0:128], in0=pss[:, 0:128],
                                    in1=maskdiag[:],
                                    op=mybir.AluOpType.add)
                            pt = sb.tile([128, SCW], BF16, name=f"pt{sc}_{h}_{tcx}",
                                         tag="pt", bufs=3)
                            nc.scalar.activation(
                                pt[:, :w], pss[:, :w],
                                mybir.ActivationFunctionType.Exp, scale=SCALE)
                            slot = tcx * NKVL + kv
                            nc.tensor.matmul(
                                po[:, off:], vsb[:, slot * HD : (slot + 1) * HD],
                                pt[:, :w],
                                start=(tcx == 0), stop=(tcx == ntc - 1))
                            nc.tensor.matmul(
                                pr[:, off:], ones[:], pt[:, :w],
                                start=(tcx == 0), stop=(tcx == ntc - 1))
                        rec = sb.tile([128, SCW], F32, name=f"rec{sc}_{h}",
                                      tag="rec", bufs=2)
                        nc.vector.reciprocal_approx_fast(out=rec[:], in_=pr[:])
                        nc.vector.tensor_tensor(
                            out=attnT[(hf, h)][:, scq * SCW : (scq + 1) * SCW],
                            in0=po[:], in1=rec[:],
                            op=mybir.AluOpType.mult)
                        draw(2)

                # wo for this half becomes available once both s-chunks done
                pending.append(wo_gen(hf))

            drain_all()
    nc.finalize()
    return nc


_NC_CACHE = None


def _get_graph():
    global _NC_CACHE
    if _NC_CACHE is None:
        _NC_CACHE = _build()
    return _NC_CACHE


_PERM = np.concatenate([np.arange(0, HD, 2), np.arange(1, HD, 2)])


def _tile_w(w):
    """[D, M*HD] -> [128, m-major kc-major 128cols] contiguous tiling."""
    d, mc = w.shape
    nm = mc // HD
    t = w.reshape(NKC, 128, nm, HD).transpose(1, 2, 0, 3)
    return np.ascontiguousarray(t.reshape(128, nm * NKC * HD))


def _tile_wo(w):
    """[NQL*HD, D] -> [128, dc-major m-major 512cols]."""
    t = w.reshape(NQL, 128, D // SCW, SCW).transpose(1, 2, 0, 3)
    return np.ascontiguousarray(t.reshape(128, (D // SCW) * NQL * SCW))


def _bf16(a):
    import ml_dtypes
    return np.ascontiguousarray(a.astype(ml_dtypes.bfloat16))


def _shard_inputs(x, freqs_cos, freqs_sin, wq, wk, wv, wo):
    """Build the 8 per-core input maps (pure numpy slicing/permutation)."""
    x = np.asarray(x, dtype=np.float32)
    wq = np.asarray(wq, dtype=np.float32)
    wk = np.asarray(wk, dtype=np.float32)
    wv = np.asarray(wv, dtype=np.float32)
    wo = np.asarray(wo, dtype=np.float32)
    cos = np.asarray(freqs_cos, dtype=np.float32)  # [S, 64]
    sin = np.asarray(freqs_sin, dtype=np.float32)

    # RoPE tables in on-chip layout [128, S]
    cos2 = np.concatenate([cos.T, cos.T], axis=0)          # [128, S]
    sin2n = np.concatenate([-sin.T, sin.T], axis=0)        # [128, S]
    cos2 = _bf16(cos2)
    sin2n = _bf16(sin2n)

    wq4 = wq.reshape(D, NH, HD)
    wk4 = wk.reshape(D, NKV, HD)
    wv4 = wv.reshape(D, NKV, HD)
    wo4 = wo.reshape(NH, HD, D)

    # x^T per batch: [128, sc-major kc-major 512]
    xts = []
    for b in range(B):
        t = x[b].reshape(NSC, SCW, NKC, 128).transpose(3, 0, 2, 1)
        xts.append(_bf16(t.reshape(128, NSC * NKC * SCW)))

    in_maps = []
    for c in range(NCORES):
        b, g = divmod(c, TPG)
        qh = slice(g * NQL, (g + 1) * NQL)
        kvh = slice(g * NKVL, (g + 1) * NKVL)
        m = {
            "xt": xts[b],
            "wq": _bf16(_tile_w(wq4[:, qh, :][:, :, _PERM].reshape(D, NQL * HD))),
            "wk": _bf16(_tile_w(wk4[:, kvh, :][:, :, _PERM].reshape(D, NKVL * HD))),
            "wv": _bf16(_tile_w(wv4[:, kvh, :].reshape(D, NKVL * HD))),
            "wo": _bf16(_tile_wo(wo4[qh].reshape(NQL * HD, D))),
            "cos2": cos2,
            "sin2n": sin2n,
        }
        in_maps.append(m)
    return in_maps


def kernel(x, start_pos, freqs_cos, freqs_sin, mask, wq, wk, wv, wo,
           cache_k, cache_v):
    x = np.asarray(x)
    in_maps = _shard_inputs(x, freqs_cos, freqs_sin, wq, wk, wv, wo)
    nc = _get_graph()
    res = run_bass_kernel_spmd(nc, in_maps, core_ids=list(range(NCORES)))
    out = np.zeros((B, S, D), dtype=np.float32)
    for b in range(B):
        acc = np.asarray(res.results[b * TPG]["out"], dtype=np.float32).copy()
        for g in range(1, TPG):
            acc += np.asarray(res.results[b * TPG + g]["out"], dtype=np.float32)
        out[b] = acc
    return out


# revision 3
# speedup vs baseline: 1.0062x; 1.0062x over previous
"""Distributed Trainium2 Bass kernel for GQA attention prefill — v2.

Problem: B=2, S=2048, D=4096, 32 q heads, 8 kv heads, head_dim=128, RoPE,
causal mask, start_pos=0.

Sharding (8 cores): DP2 over batch x TP4 over heads.  Core c = b*4 + g gets
batch b, q-heads 8g..8g+7, kv-heads 2g..2g+1, wo rows for those q-heads.
Each core computes a partial [S, D] output; the host sums the 4 partials
per batch (the row-parallel wo unshard).

v2 changes vs v1:
  - x arrives pre-transposed AND pre-cast to bf16 from the host (xt param),
    eliminating all on-device x casts/bounces/PE-transposes.
  - weights and wo arrive bf16 (host cast), halving weight DMA.
  - cos/sin RoPE tables arrive pre-built in on-chip layout [128, S] bf16.
  - causal mask add narrowed to the 128-wide diagonal block.
  - reciprocal reads the PSUM rowsum directly (no staging copy).
  - wo-projection matmuls are interleaved into the attention instruction
    stream as fillers, hiding the exp (ACT) dependency gaps.
"""

import math
from collections import deque

import numpy as np

import concourse.bass as bass  # noqa: F401  (bass types via bacc)
import concourse.mybir as mybir
from concourse import bacc
from concourse.bass_utils import run_bass_kernel_spmd
from concourse.tile import TileContext

F32 = mybir.dt.float32
BF16 = mybir.dt.bfloat16

B, S, D = 2, 2048, 4096
NH, NKV, HD = 32, 8, 128
NCORES = 8
TPG = 4                  # tensor-parallel groups
NQL = NH // TPG          # 8 local q heads
NKVL = NKV // TPG        # 2 local kv heads
SCW = 512                # s-chunk width
NSC = S // SCW           # 4 s-chunks
NKC = D // 128           # 32 contraction chunks for projections
NTC = S // 128           # 16 T-chunks (key positions)
HW = S // 2              # half width (1024)
NM = NQL + 2 * NKVL      # 12 projection m-chunks
WBLK = NKC * HD          # weight cols per m-chunk
SCALE = 1.0 / math.sqrt(HD)
NEG = -1e9


def _build():
    nc = bacc.Bacc("TRN2", target_bir_lowering=False, debug=False,
                   num_devices=NCORES)
    # x^T pre-transposed+bf16: [128, sc-major(4) x kc-major(32) x 512]
    xt_d = nc.declare_dram_parameter("xt", [128, NSC * NKC * SCW], BF16,
                                     isOutput=False)
    # weights pre-tiled bf16: [128, m-major kc-major cols]
    wq = nc.declare_dram_parameter("wq", [128, NQL * WBLK], BF16, isOutput=False)
    wk = nc.declare_dram_parameter("wk", [128, NKVL * WBLK], BF16, isOutput=False)
    wv = nc.declare_dram_parameter("wv", [128, NKVL * WBLK], BF16, isOutput=False)
    # wo pre-tiled bf16: [128, dc-major(8) x m-major(8) x 512]
    wo = nc.declare_dram_parameter("wo", [128, (D // SCW) * NQL * SCW], BF16,
                                   isOutput=False)
    # RoPE tables pre-built in on-chip layout
    cos2_d = nc.declare_dram_parameter("cos2", [128, S], BF16, isOutput=False)
    sin2n_d = nc.declare_dram_parameter("sin2n", [128, S], BF16, isOutput=False)
    out = nc.declare_dram_parameter("out", [S, D], BF16, isOutput=True)

    with TileContext(nc) as tc:
        with (
            tc.tile_pool(name="const", bufs=1) as const,
            tc.tile_pool(name="big", bufs=1) as big,
            tc.tile_pool(name="sb", bufs=3) as sb,
            tc.tile_pool(name="ps", bufs=1, space="PSUM") as ps,
        ):
            # ---- constants ------------------------------------------------
            ident = const.tile([128, 128], BF16, name="ident")
            nc.gpsimd.memset(ident[:], 0.0)
            nc.gpsimd.affine_select(
                out=ident[:], in_=ident[:],
                compare_op=mybir.AluOpType.not_equal, fill=1.0,
                base=0, pattern=[[-1, 128]], channel_multiplier=1,
            )
            ones = const.tile([128, 128], BF16, name="ones")
            nc.gpsimd.memset(ones[:], 1.0)
            # causal mask for the 128-wide diagonal block: keep where c >= p
            maskdiag = const.tile([128, 128], F32, name="maskdiag")
            nc.gpsimd.memset(maskdiag[:], 0.0)
            nc.gpsimd.affine_select(
                out=maskdiag[:], in_=maskdiag[:],
                compare_op=mybir.AluOpType.is_ge, fill=NEG,
                base=0, pattern=[[1, 128]], channel_multiplier=-1,
            )
            cos2 = const.tile([128, S], BF16, name="cos2")
            sin2n = const.tile([128, S], BF16, name="sin2n")
            nc.scalar.dma_start(out=cos2[:], in_=cos2_d[:, :])
            nc.scalar.dma_start(out=sin2n[:], in_=sin2n_d[:, :])

            ksb = big.tile([128, NKVL * S], BF16, name="ksb")
            vsb = big.tile([128, NTC * NKVL * HD], BF16, name="vsb")

            # per-(hf, head) tiles
            qt = {}
            attnT = {}

            # ---- wo filler machinery -------------------------------------
            pending = deque()

            def wo_gen(hf):
                """Output projection for s rows [hf*1024, (hf+1)*1024)."""
                for dc in range(D // SCW):
                    wot = sb.tile([128, NQL * SCW], BF16,
                                  name=f"wot{hf}_{dc}", tag="wot", bufs=2)
                    nc.scalar.dma_start(
                        out=wot[:],
                        in_=wo[:, dc * NQL * SCW : (dc + 1) * NQL * SCW])
                    for ssub in range(HW // 128):
                        pd = ps.tile([128, SCW], F32, name=f"pd{hf}_{dc}_{ssub}",
                                     tag="pp", bufs=2)
                        for kc8 in range(NQL):
                            nc.tensor.matmul(
                                pd[:],
                                attnT[(hf, kc8)][:, ssub * 128 : (ssub + 1) * 128],
                                wot[:, kc8 * SCW : (kc8 + 1) * SCW],
                                start=(kc8 == 0), stop=(kc8 == NQL - 1))
                        os_ = sb.tile([128, SCW], BF16, name=f"os{hf}_{dc}_{ssub}",
                                      tag="os", bufs=4)
                        nc.scalar.copy(out=os_[:], in_=pd[:])
                        nc.sync.dma_start(
                            out=out[hf * HW + ssub * 128 : hf * HW + (ssub + 1) * 128,
                                    dc * SCW : (dc + 1) * SCW],
                            in_=os_[:])
                        yield

            def draw(n):
                for _ in range(n):
                    while pending:
                        try:
                            next(pending[0])
                            break
                        except StopIteration:
                            pending.popleft()
                    else:
                        return

            def drain_all():
                while pending:
                    for _ in pending.popleft():
                        pass

            # ---- main schedule -------------------------------------------
            def _wsrc(m):
                if m < NQL:
                    return wq[:, m * WBLK : (m + 1) * WBLK]
                if m < NQL + NKVL:
                    return wk[:, (m - NQL) * WBLK : (m - NQL + 1) * WBLK]
                return wv[:, (m - NQL - NKVL) * WBLK
                          : (m - NQL - NKVL + 1) * WBLK]

            def do_pp(hf, m, wsl, xts, scq):
                sc = hf * 2 + scq
                ssl = slice(sc * SCW, (sc + 1) * SCW)
                if m < NQL and (hf, m) not in qt:
                    qt[(hf, m)] = sb.tile([128, HW], BF16, name=f"q{hf}_{m}",
                                          tag=f"q{m}", bufs=1)
                pp = ps.tile([128, SCW], F32, name=f"pp{hf}_{m}_{scq}",
                             tag="pp", bufs=2)
                for kc in range(NKC):
                    nc.tensor.matmul(
                        pp[:], wsl[:, kc * 128 : (kc + 1) * 128],
                        xts[scq][:, kc * SCW : (kc + 1) * SCW],
                        start=(kc == 0), stop=(kc == NKC - 1),
                    )
                if m < NQL + NKVL:
                    # RoPE -> Q tile or K store
                    if m < NQL:
                        dst = qt[(hf, m)][:, scq * SCW : (scq + 1) * SCW]
                    else:
                        kv = m - NQL
                        dst = ksb[:, kv * S + sc * SCW
                                  : kv * S + (sc + 1) * SCW]
                    t1 = sb.tile([128, SCW], BF16, name=f"t1_{hf}_{m}_{scq}",
                                 tag="t1", bufs=2)
                    t2 = sb.tile([128, SCW], BF16, name=f"t2_{hf}_{m}_{scq}",
                                 tag="t2", bufs=2)
                    nc.vector.tensor_tensor(
                        out=t1[0:64, :], in0=pp[64:128, :],
                        in1=sin2n[0:64, ssl], op=mybir.AluOpType.mult)
                    nc.vector.tensor_tensor(
                        out=t1[64:128, :], in0=pp[0:64, :],
                        in1=sin2n[64:128, ssl], op=mybir.AluOpType.mult)
                    nc.vector.tensor_tensor(
                        out=t2[:], in0=pp[:], in1=cos2[:, ssl],
                        op=mybir.AluOpType.mult)
                    nc.vector.tensor_tensor(
                        out=dst, in0=t1[:], in1=t2[:],
                        op=mybir.AluOpType.add)
                else:
                    # V: copy + PE-transpose into vsb slots
                    kv = m - NQL - NKVL
                    vts = sb.tile([128, SCW], BF16, name=f"vts{hf}_{kv}_{scq}",
                                  tag="vts", bufs=2)
                    nc.vector.tensor_copy(out=vts[:], in_=pp[:])
                    for j in range(SCW // 128):
                        pv = ps.tile([128, 128], BF16,
                                     name=f"pv{hf}_{kv}_{scq}_{j}",
                                     tag="pp", bufs=2)
                        nc.tensor.transpose(
                            pv[:], vts[:, j * 128 : (j + 1) * 128], ident[:])
                        slot = (sc * 4 + j) * NKVL + kv
                        nc.vector.tensor_copy(
                            out=vsb[:, slot * HD : (slot + 1) * HD],
                            in_=pv[:])

            def xt_load(sc):
                t = sb.tile([128, NKC * SCW], BF16, name=f"xt{sc}",
                            tag="xt", bufs=2)
                for g in range(4):
                    nc.sync.dma_start(
                        out=t[:, g * 8 * SCW : (g + 1) * 8 * SCW],
                        in_=xt_d[:, sc * NKC * SCW + g * 8 * SCW
                                 : sc * NKC * SCW + (g + 1) * 8 * SCW])
                return t

            for hf in range(2):
                morder = list(range(NQL, NM)) + list(range(NQL))
                # prefetch the two K weight slices BEFORE the bulky xT DMAs
                # (HWDGE ring is FIFO; the first matmul must not queue behind
                # 8MB of x)
                wpre = {}
                for m in morder[:2]:
                    w = sb.tile([128, WBLK], BF16, name=f"w{hf}_{m}",
                                tag="wsl", bufs=2)
                    nc.sync.dma_start(out=w[:, 0:WBLK // 2],
                                      in_=_wsrc(m)[:, 0:WBLK // 2])
                    nc.sync.dma_start(out=w[:, WBLK // 2:],
                                      in_=_wsrc(m)[:, WBLK // 2:])
                    wpre[m] = w

                # ramp order: xt(scq0) -> K projections on scq0 only ->
                # xt(scq1) -> K projections on scq1 -> rest of m-loop.
                # Keeps the PE fed from xt(scq0) while xt(scq1) streams.
                xts = {0: xt_load(hf * 2)}
                do_pp(hf, morder[0], wpre[morder[0]], xts, 0)
                do_pp(hf, morder[1], wpre[morder[1]], xts, 0)
                xts[1] = xt_load(hf * 2 + 1)
                do_pp(hf, morder[0], wpre[morder[0]], xts, 1)
                do_pp(hf, morder[1], wpre[morder[1]], xts, 1)

                for m in morder[2:]:
                    wsl = sb.tile([128, WBLK], BF16, name=f"w{hf}_{m}",
                                  tag="wsl", bufs=2)
                    nc.sync.dma_start(out=wsl[:], in_=_wsrc(m))
                    do_pp(hf, m, wsl, xts, 0)
                    do_pp(hf, m, wsl, xts, 1)

                # ---- attention for both s-chunks of this half ------------
                for scq in range(2):
                    sc = hf * 2 + scq
                    ntc = 4 * sc + 4
                    for h in range(NQL):
                        if (hf, h) not in attnT:
                            attnT[(hf, h)] = sb.tile(
                                [128, HW], BF16, name=f"at{hf}_{h}",
                                tag=f"at{h}", bufs=2)
                        kv = h // (NQL // NKVL)
                        po = ps.tile([128, SCW], F32, name=f"po{sc}_{h}",
                                     tag="po", bufs=2)
                        pr = ps.tile([128, SCW], F32, name=f"pr{sc}_{h}",
                                     tag="pr", bufs=1)
                        for tcx in range(ntc):
                            j = tcx - 4 * sc
                            off = j * 128 if j > 0 else 0
                            w = SCW - off
                            qs0 = scq * SCW + off
                            pss = ps.tile([128, SCW], F32,
                                          name=f"ps{sc}_{h}_{tcx}", tag="sc", bufs=3)
                            nc.tensor.matmul(
                                pss[:, :w],
                                ksb[:, kv * S + tcx * 128 : kv * S + (tcx + 1) * 128],
                                qt[(hf, h)][:, qs0 : qs0 + w],
                                start=True, stop=True,
                            )
                            if j >= 0:
                                nc.vector.tensor_tensor(
                                    out=pss[:, 0:128], in0=pss[:, 0:128],
                                    in1=maskdiag[:],
                                    op=mybir.AluOpType.add)
                            pt = sb.tile([128, SCW], BF16, name=f"pt{sc}_{h}_{tcx}",
                                         tag="pt", bufs=3)
                            nc.scalar.activation(
                                pt[:, :w], pss[:, :w],
                                mybir.ActivationFunctionType.Exp, scale=SCALE)
                            slot = tcx * NKVL + kv
                            nc.tensor.matmul(
                                po[:, off:], vsb[:, slot * HD : (slot + 1) * HD],
                                pt[:, :w],
                                start=(tcx == 0), stop=(tcx == ntc - 1))
                            nc.tensor.matmul(
                                pr[:, off:], ones[:], pt[:, :w],
                                start=(tcx == 0), stop=(tcx == ntc - 1))
                        rec = sb.tile([128, SCW], F32, name=f"rec{sc}_{h}",
                                      tag="rec", bufs=2)
                        nc.vector.reciprocal_approx_fast(out=rec[:], in_=pr[:])
                        nc.vector.tensor_tensor(
                            out=attnT[(hf, h)][:, scq * SCW : (scq + 1) * SCW],
                            in0=po[:], in1=rec[:],
                            op=mybir.AluOpType.mult)
                        draw(2)

                # wo for this half becomes available once both s-chunks done
                pending.append(wo_gen(hf))

            drain_all()
    nc.finalize()
    return nc


_NC_CACHE = None


def _get_graph():
    global _NC_CACHE
    if _NC_CACHE is None:
        _NC_CACHE = _build()
    return _NC_CACHE


_PERM = np.concatenate([np.arange(0, HD, 2), np.arange(1, HD, 2)])


def _tile_w(w):
    """[D, M*HD] -> [128, m-major kc-major 128cols] contiguous tiling."""
    d, mc = w.shape
    nm = mc // HD
    t = w.reshape(NKC, 128, nm, HD).transpose(1, 2, 0, 3)
    return np.ascontiguousarray(t.reshape(128, nm * NKC * HD))


def _tile_wo(w):
    """[NQL*HD, D] -> [128, dc-major m-major 512cols]."""
    t = w.reshape(NQL, 128, D // SCW, SCW).transpose(1, 2, 0, 3)
    return np.ascontiguousarray(t.reshape(128, (D // SCW) * NQL * SCW))


def _bf16(a):
    import ml_dtypes
    return np.ascontiguousarray(a.astype(ml_dtypes.bfloat16))


def _shard_inputs(x, freqs_cos, freqs_sin, wq, wk, wv, wo):
    """Build the 8 per-core input maps (pure numpy slicing/permutation)."""
    x = np.asarray(x, dtype=np.float32)
    wq = np.asarray(wq, dtype=np.float32)
    wk = np.asarray(wk, dtype=np.float32)
    wv = np.asarray(wv, dtype=np.float32)
    wo = np.asarray(wo, dtype=np.float32)
    cos = np.asarray(freqs_cos, dtype=np.float32)  # [S, 64]
    sin = np.asarray(freqs_sin, dtype=np.float32)

    # RoPE tables in on-chip layout [128, S]
    cos2 = np.concatenate([cos.T, cos.T], axis=0)          # [128, S]
    sin2n = np.concatenate([-sin.T, sin.T], axis=0)        # [128, S]
    cos2 = _bf16(cos2)
    sin2n = _bf16(sin2n)

    wq4 = wq.reshape(D, NH, HD)
    wk4 = wk.reshape(D, NKV, HD)
    wv4 = wv.reshape(D, NKV, HD)
    wo4 = wo.reshape(NH, HD, D)

    # x^T per batch: [128, sc-major kc-major 512]
    xts = []
    for b in range(B):
        t = x[b].reshape(NSC, SCW, NKC, 128).transpose(3, 0, 2, 1)
        xts.append(_bf16(t.reshape(128, NSC * NKC * SCW)))

    in_maps = []
    for c in range(NCORES):
        b, g = divmod(c, TPG)
        qh = slice(g * NQL, (g + 1) * NQL)
        kvh = slice(g * NKVL, (g + 1) * NKVL)
        m = {
            "xt": xts[b],
            "wq": _bf16(_tile_w(wq4[:, qh, :][:, :, _PERM].reshape(D, NQL * HD))),
            "wk": _bf16(_tile_w(wk4[:, kvh, :][:, :, _PERM].reshape(D, NKVL * HD))),
            "wv": _bf16(_tile_w(wv4[:, kvh, :].reshape(D, NKVL * HD))),
            "wo": _bf16(_tile_wo(wo4[qh].reshape(NQL * HD, D))),
            "cos2": cos2,
            "sin2n": sin2n,
        }
        in_maps.append(m)
    return in_maps


def kernel(x, start_pos, freqs_cos, freqs_sin, mask, wq, wk, wv, wo,
           cache_k, cache_v):
    x = np.asarray(x)
    in_maps = _shard_inputs(x, freqs_cos, freqs_sin, wq, wk, wv, wo)
    nc = _get_graph()
    res = run_bass_kernel_spmd(nc, in_maps, core_ids=list(range(NCORES)))
    out = np.zeros((B, S, D), dtype=np.float32)
    for b in range(B):
        acc = np.asarray(res.results[b * TPG]["out"], dtype=np.float32).copy()
        for g in range(1, TPG):
            acc += np.asarray(res.results[b * TPG + g]["out"], dtype=np.float32)
        out[b] = acc
    return out
